# revision 8
# baseline (speedup 1.0000x reference)
"""KNN router kernel for Trainium2 (8 NeuronCores, SPMD).

Two-phase strategy, tensor-parallel over the vocab dim V:
  - Host: L2-normalize embeddings (N=4096, D=1024) and vocab (V=32000, D)
    in float64 (correctly-rounded fp32), pad V to 32768, cast to fp16,
    transpose so D lies on the partition axis. Each of the 8 cores gets a
    4096-column vocab shard.
  - Device (per core): approximate sims = fp16(q) @ fp16(v).T in fp32 PSUM
    (single-pass matmuls, 1 cycle/row on the PE), then the TRN2 max8 /
    max_index ISA ops return the top-8 values + indices of every 512-wide
    vocab slice: 64 candidates per query row per core.
  - Host: merge the 8*64 = 512 candidates per row. fp16 rounding perturbs a
    sim by at most ~6e-5, so any row whose top-8/9 or top-1/2 margin is
    below 1.5e-4 is re-scored exactly (float64 dots over its candidates,
    which provably contain the true top-8). Rows with larger margins
    already have the exact top-8 set/argmax. Softmax (T=1) over the final
    top-8, scatter into (B, L, V), top-1 ids.
"""

import os

if "JAX_PLATFORMS" in os.environ and "axon" not in os.environ["JAX_PLATFORMS"]:
    os.environ["JAX_PLATFORMS"] = "axon," + os.environ["JAX_PLATFORMS"]

import numpy as np

_CORES = 8
_B, _L, _D, _V = 2, 2048, 1024, 32000
_NQ = _B * _L          # 4096 query rows
_VPAD = 32768          # V padded to a multiple of 8*512
_VSH = _VPAD // _CORES  # 4096 vocab rows per core
_KT = _D // 128        # 8 contraction tiles
_MT = _NQ // 128       # 32 query tiles
_NV = 512              # vocab slice width (one PSUM bank)
_NT = _VSH // _NV      # 8 slices per core
_NCAND = _NT * 8       # 64 candidates per core per row

# fp16-vs-fp32 sims deviation is < ~6e-5 (measured max 5.2e-5 over 17M
# samples); rows with a decision margin under this are exactly re-scored.
_REFINE_TH = np.float32(1.5e-4)

_cached_nc = None


def _build():
    import concourse.bacc as bacc
    import concourse.tile as tile
    from concourse import mybir

    nc = bacc.Bacc("TRN2", target_bir_lowering=False, debug=False)

    f32 = mybir.dt.float32
    f16 = mybir.dt.float16

    qth = nc.dram_tensor("qth", [_D, _NQ], f16, kind="ExternalInput").ap()
    vth = nc.dram_tensor("vth", [_D, _VSH], f16, kind="ExternalInput").ap()
    val64 = nc.dram_tensor("val64", [_NQ, _NCAND], f32, kind="ExternalOutput").ap()
    idx64 = nc.dram_tensor(
        "idx64", [_NQ, _NCAND], mybir.dt.uint32, kind="ExternalOutput"
    ).ap()

    with tile.TileContext(nc) as tc:
        with (
            tc.tile_pool(name="vT", bufs=1) as vTp,
            tc.tile_pool(name="qT", bufs=2) as qTp,
            tc.tile_pool(name="sl", bufs=4) as slp,
            tc.tile_pool(name="cand", bufs=2) as candp,
            tc.tile_pool(name="mm", bufs=6, space="PSUM") as mmp,
        ):
            # Vocab shard resident in SBUF: 8 k-slices of [128, 4096] fp16.
            # Loaded in four column groups, all k-slices of a group together,
            # so the first n-tiles' matmul operands land first.
            vTh = vTp.tile([128, _KT * _VSH], f16)
            _H = _VSH // 4
            for grp in range(4):
                cs = slice(grp * _H, (grp + 1) * _H)
                for k in range(_KT):
                    nc.sync.dma_start(
                        vTh[:, k * _VSH + grp * _H: k * _VSH + (grp + 1) * _H],
                        vth[k * 128:(k + 1) * 128, cs],
                    )
            for m in range(_MT):
                qTh = qTp.tile([128, _KT * 128], f16)
                for k in range(_KT):
                    nc.sync.dma_start(
                        qTh[:, k * 128:(k + 1) * 128],
                        qth[k * 128:(k + 1) * 128, m * 128:(m + 1) * 128],
                    )
                vout = candp.tile([128, _NCAND], f32, tag="vout")
                iout = candp.tile([128, _NCAND], mybir.dt.uint32, tag="iout")
                for n in range(_NT):
                    ps = mmp.tile([128, _NV], f32)
                    for k in range(_KT):
                        nc.tensor.matmul(
                            ps[:],
                            qTh[:, k * 128:(k + 1) * 128],
                            vTh[:, k * _VSH + n * _NV: k * _VSH + (n + 1) * _NV],
                            start=(k == 0),
                            stop=(k == _KT - 1),
                        )
                    s = slp.tile([128, _NV], f32)
                    nc.scalar.copy(s[:], ps[:])
                    nc.vector.max(vout[:, n * 8:(n + 1) * 8], s[:])
                    nc.vector.max_index(
                        iout[:, n * 8:(n + 1) * 8], vout[:, n * 8:(n + 1) * 8], s[:]
                    )
                nc.sync.dma_start(val64[m * 128:(m + 1) * 128, :], vout[:])
                nc.sync.dma_start(idx64[m * 128:(m + 1) * 128, :], iout[:])

    nc.compile()
    return nc


def _get_nc():
    global _cached_nc
    if _cached_nc is None:
        _cached_nc = _build()
    return _cached_nc


def _normalize_rows(x):
    x64 = x.astype(np.float64)
    n = np.sqrt((x64 * x64).sum(axis=-1, keepdims=True))
    return (x64 / np.maximum(n, 1e-12)).astype(np.float32)


def _prep(q, v):
    """Returns (qn, vn_padded, per-core input dicts)."""
    qn = _normalize_rows(q)
    vpad = np.zeros((_VPAD, _D), np.float32)
    vpad[:_V] = _normalize_rows(v)
    qth = np.ascontiguousarray(qn.astype(np.float16).T)     # (1024, 4096)
    vth_full = np.ascontiguousarray(vpad.astype(np.float16).T)  # (1024, 32768)
    in_maps = [
        {
            "qth": qth,
            "vth": np.ascontiguousarray(vth_full[:, c * _VSH:(c + 1) * _VSH]),
        }
        for c in range(_CORES)
    ]
    return qn, vpad, in_maps


def _prep_in_maps(q, v):
    return _prep(q, v)[2]


def kernel(embeddings, vocab_embeddings, k):
    from concourse.bass_utils import run_bass_kernel_spmd

    k = int(k)
    assert 1 <= k <= 8
    q = np.asarray(embeddings, dtype=np.float32).reshape(_NQ, _D)
    v = np.asarray(vocab_embeddings, dtype=np.float32)
    assert v.shape == (_V, _D)

    qn, vpad, in_maps = _prep(q, v)
    nc = _get_nc()
    res = run_bass_kernel_spmd(nc, in_maps, list(range(_CORES)))

    # (4096, 512) candidate values and global vocab indices
    vals = np.concatenate(
        [res.results[c]["val64"] for c in range(_CORES)], axis=1
    )
    offs = (
        np.arange(_CORES)[:, None] * _VSH
        + np.repeat(np.arange(_NT), 8)[None, :] * _NV
    ).reshape(1, -1)                                        # (1, 512) slice bases
    idxs = (
        np.concatenate(
            [res.results[c]["idx64"].astype(np.int64) for c in range(_CORES)], axis=1
        )
        + offs
    )

    # Approximate top-16 per row, ordered by (-value, index) like jax top_k.
    part = np.argpartition(-vals, 15, axis=1)[:, :16]
    pv = np.take_along_axis(vals, part, axis=1)
    pi = np.take_along_axis(idxs, part, axis=1)
    order16 = np.lexsort((pi, -pv), axis=1)
    sv = np.take_along_axis(pv, order16, axis=1)            # (4096, 16) desc
    si = np.take_along_axis(pi, order16, axis=1)

    # Rows whose top-(k)/(k+1) or top-1/2 margin could be flipped by fp16
    # error get exact re-scoring over all 512 candidates.
    qn64 = qn.astype(np.float64)
    vn64 = vpad.astype(np.float64)
    m12 = sv[:, 0] - sv[:, 1]
    mset = sv[:, k - 1] - sv[:, k]
    refine = np.where((m12 < _REFINE_TH) | (mset < _REFINE_TH))[0]

    topi = si[:, :k].copy()
    if refine.size:
        for r0 in range(0, refine.size, 64):
            rows = refine[r0:r0 + 64]
            cand = idxs[rows]                               # (r, 512)
            ex = np.einsum("rkd,rd->rk", vn64[cand], qn64[rows]).astype(np.float32)
            o = np.lexsort((cand, -ex), axis=1)[:, :k]
            topi[rows] = np.take_along_axis(cand, o, axis=1)

    # Exact values for every row's chosen top-k (fp16 approximation would
    # perturb the softmax probs by ~1e-5; this restores fp32-level values),
    # then re-order by (-value, index) to match jax.lax.top_k exactly.
    ex = np.einsum("rkd,rd->rk", vn64[topi], qn64).astype(np.float32)
    o = np.lexsort((topi, -ex), axis=1)
    topv = np.take_along_axis(ex, o, axis=1)
    topi = np.take_along_axis(topi, o, axis=1)

    # Softmax over the k candidates (temperature 1), fp32 like the reference.
    e = np.exp(topv - topv.max(axis=1, keepdims=True), dtype=np.float32)
    probs_k = e / e.sum(axis=1, keepdims=True, dtype=np.float32)

    probs = np.zeros((_NQ, _V), dtype=np.float32)
    probs[np.arange(_NQ)[:, None], topi] = probs_k
    probs = probs.reshape(_B, _L, _V)
    token_ids = topi[:, 0].astype(np.int32).reshape(_B, _L)
    return probs, token_ids


# revision 9
# speedup vs baseline: 1.0368x; 1.0368x over previous
"""KNN router kernel for Trainium2 (8 NeuronCores, SPMD).

Two-phase strategy, tensor-parallel over the vocab dim V:
  - Host: L2-normalize embeddings (N=4096, D=1024) and vocab (V=32000, D)
    in float64 (correctly-rounded fp32), pad V to 32768, cast to fp16,
    transpose so D lies on the partition axis. Each of the 8 cores gets a
    4096-column vocab shard.
  - Device (per core): approximate sims = fp16(q) @ fp16(v).T in fp32 PSUM
    (single-pass matmuls, 1 cycle/row on the PE), then the TRN2 max8 /
    max_index ISA ops return the top-8 values + indices of every 512-wide
    vocab slice: 64 candidates per query row per core.
  - Host: merge the 8*64 = 512 candidates per row. fp16 rounding perturbs a
    sim by at most ~6e-5, so any row whose top-8/9 or top-1/2 margin is
    below 1.5e-4 is re-scored exactly (float64 dots over its candidates,
    which provably contain the true top-8). Rows with larger margins
    already have the exact top-8 set/argmax. Softmax (T=1) over the final
    top-8, scatter into (B, L, V), top-1 ids.
"""

import os

if "JAX_PLATFORMS" in os.environ and "axon" not in os.environ["JAX_PLATFORMS"]:
    os.environ["JAX_PLATFORMS"] = "axon," + os.environ["JAX_PLATFORMS"]

import numpy as np

_CORES = 8
_B, _L, _D, _V = 2, 2048, 1024, 32000
_NQ = _B * _L          # 4096 query rows
_VPAD = 32768          # V padded to a multiple of 8*512
_VSH = _VPAD // _CORES  # 4096 vocab rows per core
_KT = _D // 128        # 8 contraction tiles
_MT = _NQ // 128       # 32 query tiles
_NV = 512              # vocab slice width (one PSUM bank)
_NT = _VSH // _NV      # 8 slices per core
_NCAND = _NT * 8       # 64 candidates per core per row

# fp16-vs-fp32 sims deviation is < ~6e-5 (measured max 5.2e-5 over 17M
# samples); rows with a decision margin under this are exactly re-scored.
_REFINE_TH = np.float32(1.5e-4)

_cached_nc = None


def _build():
    import concourse.bacc as bacc
    import concourse.tile as tile
    from concourse import mybir

    nc = bacc.Bacc("TRN2", target_bir_lowering=False, debug=False)

    f32 = mybir.dt.float32
    f16 = mybir.dt.float16

    qth = nc.dram_tensor("qth", [_D, _NQ], f16, kind="ExternalInput").ap()
    vth = nc.dram_tensor("vth", [_D, _VSH], f16, kind="ExternalInput").ap()
    val64 = nc.dram_tensor("val64", [_NQ, _NCAND], f32, kind="ExternalOutput").ap()
    idx64 = nc.dram_tensor(
        "idx64", [_NQ, _NCAND], mybir.dt.uint32, kind="ExternalOutput"
    ).ap()

    with tile.TileContext(nc) as tc:
        with (
            tc.tile_pool(name="vT", bufs=1) as vTp,
            tc.tile_pool(name="qT", bufs=2) as qTp,
            tc.tile_pool(name="sl", bufs=4) as slp,
            tc.tile_pool(name="cand", bufs=2) as candp,
            tc.tile_pool(name="mm", bufs=6, space="PSUM") as mmp,
        ):
            # Vocab shard resident in SBUF: 8 k-slices of [128, 4096] fp16.
            # Loaded in four column groups, all k-slices of a group together,
            # so the first n-tiles' matmul operands land first.
            vTh = vTp.tile([128, _KT * _VSH], f16)
            _H = _VSH // 4
            for grp in range(4):
                cs = slice(grp * _H, (grp + 1) * _H)
                for k in range(_KT):
                    nc.gpsimd.dma_start(
                        vTh[:, k * _VSH + grp * _H: k * _VSH + (grp + 1) * _H],
                        vth[k * 128:(k + 1) * 128, cs],
                    )
            for m in range(_MT):
                qTh = qTp.tile([128, _KT * 128], f16)
                for k in range(_KT):
                    nc.sync.dma_start(
                        qTh[:, k * 128:(k + 1) * 128],
                        qth[k * 128:(k + 1) * 128, m * 128:(m + 1) * 128],
                    )
                vout = candp.tile([128, _NCAND], f32, tag="vout")
                iout = candp.tile([128, _NCAND], mybir.dt.uint32, tag="iout")
                for n in range(_NT):
                    ps = mmp.tile([128, _NV], f32)
                    for k in range(_KT):
                        nc.tensor.matmul(
                            ps[:],
                            qTh[:, k * 128:(k + 1) * 128],
                            vTh[:, k * _VSH + n * _NV: k * _VSH + (n + 1) * _NV],
                            start=(k == 0),
                            stop=(k == _KT - 1),
                        )
                    s = slp.tile([128, _NV], f32)
                    nc.scalar.copy(s[:], ps[:])
                    nc.vector.max(vout[:, n * 8:(n + 1) * 8], s[:])
                    nc.vector.max_index(
                        iout[:, n * 8:(n + 1) * 8], vout[:, n * 8:(n + 1) * 8], s[:]
                    )
                nc.sync.dma_start(val64[m * 128:(m + 1) * 128, :], vout[:])
                nc.sync.dma_start(idx64[m * 128:(m + 1) * 128, :], iout[:])

    nc.compile()
    return nc


def _get_nc():
    global _cached_nc
    if _cached_nc is None:
        _cached_nc = _build()
    return _cached_nc


def _normalize_rows(x):
    x64 = x.astype(np.float64)
    n = np.sqrt((x64 * x64).sum(axis=-1, keepdims=True))
    return (x64 / np.maximum(n, 1e-12)).astype(np.float32)


def _prep(q, v):
    """Returns (qn, vn_padded, per-core input dicts)."""
    qn = _normalize_rows(q)
    vpad = np.zeros((_VPAD, _D), np.float32)
    vpad[:_V] = _normalize_rows(v)
    qth = np.ascontiguousarray(qn.astype(np.float16).T)     # (1024, 4096)
    vth_full = np.ascontiguousarray(vpad.astype(np.float16).T)  # (1024, 32768)
    in_maps = [
        {
            "qth": qth,
            "vth": np.ascontiguousarray(vth_full[:, c * _VSH:(c + 1) * _VSH]),
        }
        for c in range(_CORES)
    ]
    return qn, vpad, in_maps


def _prep_in_maps(q, v):
    return _prep(q, v)[2]


def kernel(embeddings, vocab_embeddings, k):
    from concourse.bass_utils import run_bass_kernel_spmd

    k = int(k)
    assert 1 <= k <= 8
    q = np.asarray(embeddings, dtype=np.float32).reshape(_NQ, _D)
    v = np.asarray(vocab_embeddings, dtype=np.float32)
    assert v.shape == (_V, _D)

    qn, vpad, in_maps = _prep(q, v)
    nc = _get_nc()
    res = run_bass_kernel_spmd(nc, in_maps, list(range(_CORES)))

    # (4096, 512) candidate values and global vocab indices
    vals = np.concatenate(
        [res.results[c]["val64"] for c in range(_CORES)], axis=1
    )
    offs = (
        np.arange(_CORES)[:, None] * _VSH
        + np.repeat(np.arange(_NT), 8)[None, :] * _NV
    ).reshape(1, -1)                                        # (1, 512) slice bases
    idxs = (
        np.concatenate(
            [res.results[c]["idx64"].astype(np.int64) for c in range(_CORES)], axis=1
        )
        + offs
    )

    # Approximate top-16 per row, ordered by (-value, index) like jax top_k.
    part = np.argpartition(-vals, 15, axis=1)[:, :16]
    pv = np.take_along_axis(vals, part, axis=1)
    pi = np.take_along_axis(idxs, part, axis=1)
    order16 = np.lexsort((pi, -pv), axis=1)
    sv = np.take_along_axis(pv, order16, axis=1)            # (4096, 16) desc
    si = np.take_along_axis(pi, order16, axis=1)

    # Rows whose top-(k)/(k+1) or top-1/2 margin could be flipped by fp16
    # error get exact re-scoring over all 512 candidates.
    qn64 = qn.astype(np.float64)
    vn64 = vpad.astype(np.float64)
    m12 = sv[:, 0] - sv[:, 1]
    mset = sv[:, k - 1] - sv[:, k]
    refine = np.where((m12 < _REFINE_TH) | (mset < _REFINE_TH))[0]

    topi = si[:, :k].copy()
    if refine.size:
        for r0 in range(0, refine.size, 64):
            rows = refine[r0:r0 + 64]
            cand = idxs[rows]                               # (r, 512)
            ex = np.einsum("rkd,rd->rk", vn64[cand], qn64[rows]).astype(np.float32)
            o = np.lexsort((cand, -ex), axis=1)[:, :k]
            topi[rows] = np.take_along_axis(cand, o, axis=1)

    # Exact values for every row's chosen top-k (fp16 approximation would
    # perturb the softmax probs by ~1e-5; this restores fp32-level values),
    # then re-order by (-value, index) to match jax.lax.top_k exactly.
    ex = np.einsum("rkd,rd->rk", vn64[topi], qn64).astype(np.float32)
    o = np.lexsort((topi, -ex), axis=1)
    topv = np.take_along_axis(ex, o, axis=1)
    topi = np.take_along_axis(topi, o, axis=1)

    # Softmax over the k candidates (temperature 1), fp32 like the reference.
    e = np.exp(topv - topv.max(axis=1, keepdims=True), dtype=np.float32)
    probs_k = e / e.sum(axis=1, keepdims=True, dtype=np.float32)

    probs = np.zeros((_NQ, _V), dtype=np.float32)
    probs[np.arange(_NQ)[:, None], topi] = probs_k
    probs = probs.reshape(_B, _L, _V)
    token_ids = topi[:, 0].astype(np.int32).reshape(_B, _L)
    return probs, token_ids
